# revision 12
# baseline (speedup 1.0000x reference)
"""GridMask forward: y = x * mask(cell_active, off_i, off_j, d, apply_flag).

Distribution: pure data parallel over the batch axis — each of the 8
NeuronCores gets a [16, 3, 384, 384] shard of x plus the (replicated)
precomputed [384, 384] mask, and does the elementwise multiply on-device.
The mask itself is a function of the tiny 8x8 grid parameters, computed
host-side in numpy (exact mirror of the reference semantics).

Device kernel (per core): x viewed as 144 blocks of [128, 384]; tiles of
12 blocks (= 4 full images, so the mask pattern per tile is identical)
are DMA'd in as [128, 4608], multiplied in-place on the vector engine by
an SBUF-resident mask replica, and DMA'd back out.
"""

import numpy as np

_R = 0.6
_B, _C, _H, _W = 128, 3, 384, 384
_NCORES = 8
_BPC = _B // _NCORES          # batches per core
_P = 128                      # SBUF partitions
_RB = _H // _P                # row blocks per image
_NBLK = _BPC * _C * _RB       # [128, 384] blocks per core
_GBLK = 6                     # blocks per tile (multiple of _RB)
_NT = _NBLK // _GBLK

_nc_cache = None


def _host_mask(cell_active, off_i, off_j, d, h, w, apply_flag):
    if int(apply_flag) <= 0:
        return np.ones((h, w), dtype=np.float32)
    l = int(d * _R)
    starts_i = np.arange(0, h, d, dtype=np.int64)
    starts_j = np.arange(0, w, d, dtype=np.int64)
    i_pos = np.clip(starts_i[:, None] + (off_i.astype(np.int64) - 2), 0, h - l)
    j_pos = np.clip(starts_j[None, :] + (off_j.astype(np.int64) - 2), 0, w - l)
    rows = np.arange(h, dtype=np.int64)
    cols = np.arange(w, dtype=np.int64)
    row_in = (rows >= i_pos[..., None]) & (rows < i_pos[..., None] + l)  # [gh,gw,h]
    col_in = (cols >= j_pos[..., None]) & (cols < j_pos[..., None] + l)  # [gh,gw,w]
    act = cell_active[..., None] > 0
    covered = ((row_in & act)[:, :, :, None] & col_in[:, :, None, :]).any(axis=(0, 1))
    return np.where(covered, np.float32(0), np.float32(1))


def _build_bass():
    global _nc_cache
    if _nc_cache is not None:
        return _nc_cache
    import concourse.bacc as bacc
    import concourse.mybir as mybir
    from concourse.mybir import AluOpType
    from concourse.tile import TileContext

    f32 = mybir.dt.float32
    nc = bacc.Bacc()
    x = nc.dram_tensor("x", [_NBLK, _P, _W], f32, kind="ExternalInput")
    m = nc.dram_tensor("mask", [_RB, _P, _W], f32, kind="ExternalInput")
    y = nc.dram_tensor("y", [_NBLK, _P, _W], f32, kind="ExternalOutput")
    with TileContext(nc) as tc:
        with (
            tc.tile_pool(name="mrep", bufs=1) as mpool,
            tc.tile_pool(name="xb", bufs=6) as xpool,
            tc.tile_pool(name="yb", bufs=6) as ypool,
        ):
            # Load the [3, 128, 384] mask once and replicate it on-chip to
            # cover a full tile (doubling copy on the DVE).
            mrep = mpool.tile([_P, _GBLK, _W], f32)
            nc.sync.dma_start(
                out=mrep[:, 0:_RB, :], in_=m[:].rearrange("r p w -> p r w")
            )
            mflat = mrep[:].rearrange("p n w -> p (n w)")
            rw = _RB * _W
            for rep in range(1, _GBLK // _RB):
                nc.vector.tensor_copy(mflat[:, rep * rw : (rep + 1) * rw], mflat[:, 0:rw])
            for t in range(_NT):
                xt = xpool.tile([_P, _GBLK, _W], f32)
                yt = ypool.tile([_P, _GBLK, _W], f32)
                nc.sync.dma_start(
                    out=xt[:],
                    in_=x[t * _GBLK : (t + 1) * _GBLK].rearrange("n p w -> p n w"),
                )
                xt2 = xt[:].rearrange("p n w -> p (n w)")
                yt2 = yt[:].rearrange("p n w -> p (n w)")
                nc.vector.tensor_tensor(yt2, xt2, mflat, AluOpType.mult)
                # Stores go on the ACT HWDGE ring so they don't serialize
                # behind loads in the SP ring's descriptor FIFO.
                nc.scalar.dma_start(
                    out=y[t * _GBLK : (t + 1) * _GBLK].rearrange("n p w -> p n w"),
                    in_=yt[:],
                )
    nc.finalize()
    _nc_cache = nc
    return nc


def run_device(x, mask, trace=False, **spmd_kwargs):
    """Run the sharded device multiply. x: [128,3,384,384] f32 contiguous,
    mask: [384,384] f32. Returns (y [128,3,384,384], BassKernelResults)."""
    from concourse.bass_utils import run_bass_kernel_spmd

    nc = _build_bass()
    xv = x.reshape(_NCORES, _NBLK, _P, _W)
    mview = np.ascontiguousarray(mask.reshape(_RB, _P, _W))
    in_maps = [{"x": xv[c], "mask": mview} for c in range(_NCORES)]
    res = run_bass_kernel_spmd(
        nc, in_maps, core_ids=list(range(_NCORES)), trace=trace, **spmd_kwargs
    )
    y = np.stack([res.results[c]["y"] for c in range(_NCORES)], axis=0)
    return y.reshape(_B, _C, _H, _W), res


def kernel(x, cell_active, off_i, off_j, d, apply_flag):
    x = np.ascontiguousarray(np.asarray(x), dtype=np.float32)
    mask = _host_mask(
        np.asarray(cell_active), np.asarray(off_i), np.asarray(off_j),
        int(d), _H, _W, int(apply_flag),
    )
    y, _ = run_device(x, mask)
    return y


# revision 14
# speedup vs baseline: 1.1306x; 1.1306x over previous
"""GridMask forward: y = x * mask(cell_active, off_i, off_j, d, apply_flag).

Distribution: pure data parallel over the batch axis — each of the 8
NeuronCores gets a [16, 3, 384, 384] shard of x plus the (replicated)
precomputed [384, 384] mask, and does the elementwise multiply on-device.
The mask itself is a function of the tiny 8x8 grid parameters, computed
host-side in numpy (exact mirror of the reference semantics).

Device kernel (per core): x viewed as 144 blocks of [128, 384]; tiles of
12 blocks (= 4 full images, so the mask pattern per tile is identical)
are DMA'd in as [128, 4608], multiplied in-place on the vector engine by
an SBUF-resident mask replica, and DMA'd back out.
"""

import numpy as np

_R = 0.6
_B, _C, _H, _W = 128, 3, 384, 384
_NCORES = 8
_BPC = _B // _NCORES          # batches per core
_P = 128                      # SBUF partitions
_RB = _H // _P                # row blocks per image
_NBLK = _BPC * _C * _RB       # [128, 384] blocks per core
_GBLK = 12                    # blocks per tile (multiple of _RB)
_NT = _NBLK // _GBLK

_nc_cache = None


def _host_mask(cell_active, off_i, off_j, d, h, w, apply_flag):
    if int(apply_flag) <= 0:
        return np.ones((h, w), dtype=np.float32)
    l = int(d * _R)
    starts_i = np.arange(0, h, d, dtype=np.int64)
    starts_j = np.arange(0, w, d, dtype=np.int64)
    i_pos = np.clip(starts_i[:, None] + (off_i.astype(np.int64) - 2), 0, h - l)
    j_pos = np.clip(starts_j[None, :] + (off_j.astype(np.int64) - 2), 0, w - l)
    rows = np.arange(h, dtype=np.int64)
    cols = np.arange(w, dtype=np.int64)
    row_in = (rows >= i_pos[..., None]) & (rows < i_pos[..., None] + l)  # [gh,gw,h]
    col_in = (cols >= j_pos[..., None]) & (cols < j_pos[..., None] + l)  # [gh,gw,w]
    act = cell_active[..., None] > 0
    covered = ((row_in & act)[:, :, :, None] & col_in[:, :, None, :]).any(axis=(0, 1))
    return np.where(covered, np.float32(0), np.float32(1))


def _build_bass():
    global _nc_cache
    if _nc_cache is not None:
        return _nc_cache
    import concourse.bacc as bacc
    import concourse.mybir as mybir
    from concourse.mybir import AluOpType
    from concourse.tile import TileContext

    f32 = mybir.dt.float32
    nc = bacc.Bacc()
    x = nc.dram_tensor("x", [_NBLK, _P, _W], f32, kind="ExternalInput")
    m = nc.dram_tensor("mask", [_RB, _P, _W], f32, kind="ExternalInput")
    y = nc.dram_tensor("y", [_NBLK, _P, _W], f32, kind="ExternalOutput")
    with TileContext(nc) as tc:
        with (
            tc.tile_pool(name="mrep", bufs=1) as mpool,
            tc.tile_pool(name="xb", bufs=4) as xpool,
            tc.tile_pool(name="yb", bufs=4) as ypool,
        ):
            # Load the [3, 128, 384] mask once and replicate it on-chip to
            # cover a full tile (doubling copy on the DVE).
            mrep = mpool.tile([_P, _GBLK, _W], f32)
            nc.sync.dma_start(
                out=mrep[:, 0:_RB, :], in_=m[:].rearrange("r p w -> p r w")
            )
            mflat = mrep[:].rearrange("p n w -> p (n w)")
            rw = _RB * _W
            for rep in range(1, _GBLK // _RB):
                nc.vector.tensor_copy(mflat[:, rep * rw : (rep + 1) * rw], mflat[:, 0:rw])
            for t in range(_NT):
                xt = xpool.tile([_P, _GBLK, _W], f32)
                yt = ypool.tile([_P, _GBLK, _W], f32)
                nc.sync.dma_start(
                    out=xt[:],
                    in_=x[t * _GBLK : (t + 1) * _GBLK].rearrange("n p w -> p n w"),
                )
                xt2 = xt[:].rearrange("p n w -> p (n w)")
                yt2 = yt[:].rearrange("p n w -> p (n w)")
                nc.vector.tensor_tensor(yt2, xt2, mflat, AluOpType.mult)
                # Stores go on the ACT HWDGE ring so they don't serialize
                # behind loads in the SP ring's descriptor FIFO.
                nc.scalar.dma_start(
                    out=y[t * _GBLK : (t + 1) * _GBLK].rearrange("n p w -> p n w"),
                    in_=yt[:],
                )
    nc.finalize()
    _nc_cache = nc
    return nc


def run_device(x, mask, trace=False, **spmd_kwargs):
    """Run the sharded device multiply. x: [128,3,384,384] f32 contiguous,
    mask: [384,384] f32. Returns (y [128,3,384,384], BassKernelResults)."""
    from concourse.bass_utils import run_bass_kernel_spmd

    nc = _build_bass()
    xv = x.reshape(_NCORES, _NBLK, _P, _W)
    mview = np.ascontiguousarray(mask.reshape(_RB, _P, _W))
    in_maps = [{"x": xv[c], "mask": mview} for c in range(_NCORES)]
    res = run_bass_kernel_spmd(
        nc, in_maps, core_ids=list(range(_NCORES)), trace=trace, **spmd_kwargs
    )
    y = np.stack([res.results[c]["y"] for c in range(_NCORES)], axis=0)
    return y.reshape(_B, _C, _H, _W), res


def kernel(x, cell_active, off_i, off_j, d, apply_flag):
    x = np.ascontiguousarray(np.asarray(x), dtype=np.float32)
    mask = _host_mask(
        np.asarray(cell_active), np.asarray(off_i), np.asarray(off_j),
        int(d), _H, _W, int(apply_flag),
    )
    y, _ = run_device(x, mask)
    return y


# revision 15
# speedup vs baseline: 1.1994x; 1.0608x over previous
"""GridMask forward: y = x * mask(cell_active, off_i, off_j, d, apply_flag).

Distribution: pure data parallel over the batch axis — each of the 8
NeuronCores gets a [16, 3, 384, 384] shard of x plus the (replicated)
precomputed [384, 384] mask, and does the elementwise multiply on-device.
The mask itself is a function of the tiny 8x8 grid parameters, computed
host-side in numpy (exact mirror of the reference semantics).

Device kernel (per core): x viewed as 144 blocks of [128, 384]; tiles of
12 blocks (= 4 full images, so the mask pattern per tile is identical)
are DMA'd in as [128, 4608], multiplied in-place on the vector engine by
an SBUF-resident mask replica, and DMA'd back out.
"""

import numpy as np

_R = 0.6
_B, _C, _H, _W = 128, 3, 384, 384
_NCORES = 8
_BPC = _B // _NCORES          # batches per core
_P = 128                      # SBUF partitions
_RB = _H // _P                # row blocks per image
_NBLK = _BPC * _C * _RB       # [128, 384] blocks per core
_GBLK = 12                    # blocks per tile (multiple of _RB)
_NT = _NBLK // _GBLK

_nc_cache = None


def _host_mask(cell_active, off_i, off_j, d, h, w, apply_flag):
    if int(apply_flag) <= 0:
        return np.ones((h, w), dtype=np.float32)
    l = int(d * _R)
    starts_i = np.arange(0, h, d, dtype=np.int64)
    starts_j = np.arange(0, w, d, dtype=np.int64)
    i_pos = np.clip(starts_i[:, None] + (off_i.astype(np.int64) - 2), 0, h - l)
    j_pos = np.clip(starts_j[None, :] + (off_j.astype(np.int64) - 2), 0, w - l)
    rows = np.arange(h, dtype=np.int64)
    cols = np.arange(w, dtype=np.int64)
    row_in = (rows >= i_pos[..., None]) & (rows < i_pos[..., None] + l)  # [gh,gw,h]
    col_in = (cols >= j_pos[..., None]) & (cols < j_pos[..., None] + l)  # [gh,gw,w]
    act = cell_active[..., None] > 0
    covered = ((row_in & act)[:, :, :, None] & col_in[:, :, None, :]).any(axis=(0, 1))
    return np.where(covered, np.float32(0), np.float32(1))


def _build_bass():
    global _nc_cache
    if _nc_cache is not None:
        return _nc_cache
    import concourse.bacc as bacc
    import concourse.mybir as mybir
    from concourse.mybir import AluOpType
    from concourse.tile import TileContext

    f32 = mybir.dt.float32
    nc = bacc.Bacc()
    x = nc.dram_tensor("x", [_NBLK, _P, _W], f32, kind="ExternalInput")
    m = nc.dram_tensor("mask", [_RB, _P, _W], f32, kind="ExternalInput")
    y = nc.dram_tensor("y", [_NBLK, _P, _W], f32, kind="ExternalOutput")
    with TileContext(nc) as tc:
        with (
            tc.tile_pool(name="mrep", bufs=1) as mpool,
            tc.tile_pool(name="xb", bufs=4) as xpool,
            tc.tile_pool(name="yb", bufs=4) as ypool,
        ):
            # Load the [3, 128, 384] mask once and replicate it on-chip to
            # cover a full tile (doubling copy on the DVE).
            mrep = mpool.tile([_P, _GBLK, _W], f32)
            nc.sync.dma_start(
                out=mrep[:, 0:_RB, :], in_=m[:].rearrange("r p w -> p r w")
            )
            mflat = mrep[:].rearrange("p n w -> p (n w)")
            rw = _RB * _W
            for rep in range(1, _GBLK // _RB):
                nc.vector.tensor_copy(mflat[:, rep * rw : (rep + 1) * rw], mflat[:, 0:rw])
            # Variable tile sizes (in blocks, multiples of _RB): small tiles
            # at the start so the first store begins early, big 2.25 MiB
            # tiles in the middle for DMA efficiency, small tiles at the
            # end to shorten the serial load->mul->store tail.
            sizes = [3, 6] + [12] * 10 + [6, 6, 3]
            assert sum(sizes) == _NBLK and all(s % _RB == 0 for s in sizes)
            off = 0
            for s in sizes:
                xt = xpool.tile([_P, _GBLK, _W], f32, tag="xb")
                yt = ypool.tile([_P, _GBLK, _W], f32, tag="yb")
                nc.sync.dma_start(
                    out=xt[:, 0:s, :],
                    in_=x[off : off + s].rearrange("n p w -> p n w"),
                )
                xt2 = xt[:].rearrange("p n w -> p (n w)")
                yt2 = yt[:].rearrange("p n w -> p (n w)")
                nc.vector.tensor_tensor(
                    yt2[:, 0 : s * _W], xt2[:, 0 : s * _W], mflat[:, 0 : s * _W],
                    AluOpType.mult,
                )
                # Stores go on the ACT HWDGE ring so they don't serialize
                # behind loads in the SP ring's descriptor FIFO.
                nc.scalar.dma_start(
                    out=y[off : off + s].rearrange("n p w -> p n w"),
                    in_=yt[:, 0:s, :],
                )
                off += s
    nc.finalize()
    _nc_cache = nc
    return nc


def run_device(x, mask, trace=False, **spmd_kwargs):
    """Run the sharded device multiply. x: [128,3,384,384] f32 contiguous,
    mask: [384,384] f32. Returns (y [128,3,384,384], BassKernelResults)."""
    from concourse.bass_utils import run_bass_kernel_spmd

    nc = _build_bass()
    xv = x.reshape(_NCORES, _NBLK, _P, _W)
    mview = np.ascontiguousarray(mask.reshape(_RB, _P, _W))
    in_maps = [{"x": xv[c], "mask": mview} for c in range(_NCORES)]
    res = run_bass_kernel_spmd(
        nc, in_maps, core_ids=list(range(_NCORES)), trace=trace, **spmd_kwargs
    )
    y = np.stack([res.results[c]["y"] for c in range(_NCORES)], axis=0)
    return y.reshape(_B, _C, _H, _W), res


def kernel(x, cell_active, off_i, off_j, d, apply_flag):
    x = np.ascontiguousarray(np.asarray(x), dtype=np.float32)
    mask = _host_mask(
        np.asarray(cell_active), np.asarray(off_i), np.asarray(off_j),
        int(d), _H, _W, int(apply_flag),
    )
    y, _ = run_device(x, mask)
    return y
